# revision 1
# baseline (speedup 1.0000x reference)
"""Trainium2 kernel for nn_GUP_4105988735544 (gnn_message_passing).

Scene-parallel sharding: B=32 scenes split across 8 NeuronCores (4 each);
the small 128-dim weights are replicated on every core. Each core runs the
fused attention + LayerNorm + FFN block for its scenes; outputs are
gathered back to a single full-shape array.
"""

import numpy as np
import jax
import jax.numpy as jnp
from jax.sharding import Mesh, NamedSharding, PartitionSpec as P

B, M, AQ, LK, D, H = 32, 6, 128, 128, 512, 8  # placeholder, fixed below
B, M, AQ, LK, D, H = 32, 6, 128, 512, 128, 8
HD = D // H
LN_EPS = 1e-5
N_CORES = 8

_devices = jax.devices()[:N_CORES]
_mesh = Mesh(np.array(_devices), ("x",))
_batch_sh = NamedSharding(_mesh, P("x"))
_repl_sh = NamedSharding(_mesh, P())


def _layer_norm(x, g, b):
    mu = jnp.mean(x, axis=-1, keepdims=True)
    var = jnp.var(x, axis=-1, keepdims=True)
    return (x - mu) * jax.lax.rsqrt(var + LN_EPS) * g + b


def _block(query, key_value, attn_mask, Wq, bq, Wk, Wv, bv, Wo, bo,
           mlp_w1, mlp_b1, mlp_ln_g, mlp_ln_b, mlp_w2, mlp_b2,
           ln1_g, ln1_b, ln2_g, ln2_b):
    b = query.shape[0]
    q = (query @ Wq.T + bq).reshape(b, M, AQ, H, HD)
    k = (key_value @ Wk.T).reshape(b, M, LK, H, HD)
    v = (key_value @ Wv.T + bv).reshape(b, M, LK, H, HD)
    scale = 1.0 / jnp.sqrt(jnp.float32(HD))
    scores = jnp.einsum("bmqhd,bmkhd->bhmqk", q * scale, k)
    ext_mask = (1.0 - attn_mask) * -10000.0
    scores = scores + ext_mask[:, None, None, :, :]
    probs = jax.nn.softmax(scores, axis=-1)
    ctx = jnp.einsum("bhmqk,bmkhd->bmqhd", probs, v).reshape(b, M, AQ, D)
    attn_out = ctx @ Wo.T + bo
    x = _layer_norm(attn_out + query, ln1_g, ln1_b)
    h = jax.nn.relu(_layer_norm(x @ mlp_w1.T + mlp_b1, mlp_ln_g, mlp_ln_b))
    ffn = h @ mlp_w2.T + mlp_b2
    return _layer_norm(ffn + x, ln2_g, ln2_b)


_BATCH_ARGS = ("query", "key_value", "attn_mask")

_in_shardings = None
_jitted = None


def _get_jitted():
    global _jitted
    if _jitted is None:
        import functools
        names = ["query", "key_value", "attn_mask", "Wq", "bq", "Wk", "Wv",
                 "bv", "Wo", "bo", "mlp_w1", "mlp_b1", "mlp_ln_g", "mlp_ln_b",
                 "mlp_w2", "mlp_b2", "ln1_g", "ln1_b", "ln2_g", "ln2_b"]
        shardings = tuple(_batch_sh if n in _BATCH_ARGS else _repl_sh
                          for n in names)
        _jitted = jax.jit(_block, in_shardings=shardings,
                          out_shardings=_batch_sh)
    return _jitted


def kernel(**inputs) -> np.ndarray:
    fn = _get_jitted()
    names = ["query", "key_value", "attn_mask", "Wq", "bq", "Wk", "Wv",
             "bv", "Wo", "bo", "mlp_w1", "mlp_b1", "mlp_ln_g", "mlp_ln_b",
             "mlp_w2", "mlp_b2", "ln1_g", "ln1_b", "ln2_g", "ln2_b"]
    args = []
    for n in names:
        a = jnp.asarray(np.asarray(inputs[n], dtype=np.float32))
        sh = _batch_sh if n in _BATCH_ARGS else _repl_sh
        args.append(jax.device_put(a, sh))
    out = fn(*args)
    return np.asarray(jax.device_get(out), dtype=np.float32)
